# revision 18
# baseline (speedup 1.0000x reference)
# Trainium2 Bass kernel for nn_EpisodeMultiheadAttentionBlock.
# B=8, S=1024, E=1024, H=8 heads, HD=128. Data-parallel over batch: core b
# computes batch element b. Self-contained: only needs /opt/trn_rl_repo on path.
#
# v3: fp8(e4m3) DoubleRow matmuls for QKV/Wo/y-side gates and attention AV/den
# (weights pre-scaled by powers of 2, dequantized via activation scales);
# host-precomputed x^T and RoPE cos/sin tables; bf16 where precision demands
# it (scores, x-side gate matmuls, rope rotation). Single long-lived PSUM
# pool (no phase barriers), DMA loads ordered for prefetch, elementwise work
# spread across DVE/Act/Pool.
import sys
import numpy as np

sys.path.insert(0, "/opt/trn_rl_repo")

import ml_dtypes  # noqa: E402
import concourse.bass as bass  # noqa: E402
import concourse.mybir as mybir  # noqa: E402
import concourse.tile as tile  # noqa: E402
from concourse import bacc  # noqa: E402
from concourse import bass_utils  # noqa: E402

B, S, E, H = 8, 1024, 1024, 8
HD = E // H  # 128
NT = E // 128  # 8 e-tiles / s-tiles
NP = NT // 2  # 4 DoubleRow k-tile pairs
NC = 8  # cores
BF16 = mybir.dt.bfloat16
F32 = mybir.dt.float32
FP8 = mybir.dt.float8e4
AF = mybir.ActivationFunctionType
DR = mybir.MatmulPerfMode.DoubleRow
ALU = mybir.AluOpType
NPBF16 = ml_dtypes.bfloat16
NPFP8 = ml_dtypes.float8_e4m3

WS = 32.0  # weight pre-scale for fp8/bf16 weights
YS = 8.0   # y stored as 8*y in fp8
CS = 16.0  # ctx stored as 16*ctx in fp8

_COMPILED = {}


def _build(share_qk: bool):
    nc = bacc.Bacc("TRN2", target_bir_lowering=False, debug=False, num_devices=NC)

    # ---- DRAM tensors -------------------------------------------------------
    xb_d = nc.dram_tensor("xb", [S, E], F32, kind="ExternalInput")
    xtb_d = nc.dram_tensor("xtb", [128, NT * S], BF16, kind="ExternalInput")
    xt8_d = nc.dram_tensor("xt8", [128, NT * S], FP8, kind="ExternalInput")
    w8_d = {
        nm: nc.dram_tensor(nm, [128, NT * E], FP8, kind="ExternalInput")
        for nm in ("Wq", "Wk", "Wv", "Wo", "Wyr", "Wyz", "Wyg")
    }
    wb_d = {
        nm: nc.dram_tensor(nm, [128, NT * E], BF16, kind="ExternalInput")
        for nm in ("Wxr", "Wxz", "Wxg")
    }
    bq_d = nc.dram_tensor("bq", [128, NT], F32, kind="ExternalInput")
    bk_d = nc.dram_tensor("bk", [128, NT], F32, kind="ExternalInput")
    bo8_d = nc.dram_tensor("bo8", [128, NT], F32, kind="ExternalInput")
    bv_row_d = nc.dram_tensor("bv_row", [1, E], BF16, kind="ExternalInput")   # 32*bv
    bxz_row_d = nc.dram_tensor("bxz_row", [1, E], BF16, kind="ExternalInput")  # 32*bxz
    cosq_d = nc.dram_tensor("cosq", [128, NT * S], BF16, kind="ExternalInput")
    sinq_d = nc.dram_tensor("sinq", [128, NT * S], BF16, kind="ExternalInput")
    if not share_qk:
        cosk_d = nc.dram_tensor("cosk", [128, NT * S], BF16, kind="ExternalInput")
        sink_d = nc.dram_tensor("sink", [128, NT * S], BF16, kind="ExternalInput")
    ones8_d = nc.dram_tensor("ones8", [128, 256], FP8, kind="ExternalInput")
    out_d = nc.dram_tensor("out", [S, E], F32, kind="ExternalOutput")

    SCALE = 1.0 / float(np.sqrt(HD))

    def r3(ap):
        return ap.rearrange("p (t s) -> p t s", t=NT)

    with tile.TileContext(nc) as tc:
      from contextlib import ExitStack

      with ExitStack() as top:
        res = top.enter_context(tc.tile_pool(name="res", bufs=1))
        consts = top.enter_context(tc.tile_pool(name="consts", bufs=1))
        wp8 = top.enter_context(tc.tile_pool(name="wp8", bufs=4))
        wpb = top.enter_context(tc.tile_pool(name="wpb", bufs=2))
        psum = top.enter_context(tc.tile_pool(name="psum", bufs=1, space="PSUM"))

        def load_w8(nm):
            t = wp8.tile([128, NT, E], FP8, tag="W8", name=f"w_{nm}")
            nc.sync.dma_start(out=t, in_=w8_d[nm].ap().rearrange("p (t e) -> p t e", t=NT))
            return t

        def load_wb(nm):
            t = wpb.tile([128, NT, E], BF16, tag="Wb", name=f"w_{nm}")
            nc.sync.dma_start(out=t, in_=wb_d[nm].ap().rearrange("p (t e) -> p t e", t=NT))
            return t

        # ---------------- loads in prefetch order ---------------------------
        xT8 = res.tile([128, NT, S], FP8, tag="xT8")
        nc.sync.dma_start(out=xT8, in_=r3(xt8_d.ap()))
        wv_sb = load_w8("Wv")
        wq_sb = load_w8("Wq")
        xTb = res.tile([128, NT, S], BF16, tag="xTb")
        nc.sync.dma_start(out=xTb, in_=r3(xtb_d.ap()))
        wk_sb = load_w8("Wk")

        bq_sb = consts.tile([128, NT], F32, tag="bq")
        nc.sync.dma_start(out=bq_sb, in_=bq_d.ap())
        bk_sb = consts.tile([128, NT], F32, tag="bk")
        nc.sync.dma_start(out=bk_sb, in_=bk_d.ap())
        bo8_sb = consts.tile([128, NT], F32, tag="bo8")
        nc.sync.dma_start(out=bo8_sb, in_=bo8_d.ap())
        bv_row = consts.tile([1, E], BF16, tag="bv_row")
        nc.sync.dma_start(out=bv_row, in_=bv_row_d.ap())
        bxz_row = consts.tile([1, E], BF16, tag="bxz_row")
        nc.sync.dma_start(out=bxz_row, in_=bxz_row_d.ap())
        ones8 = consts.tile([128, 2, 128], FP8, tag="ones8")  # value 1/CS
        nc.sync.dma_start(out=ones8, in_=ones8_d.ap().rearrange("p (a b) -> p a b", a=2))
        ones1_b = consts.tile([1, 128], BF16, tag="ones1_b")
        nc.vector.memset(ones1_b, 1.0)

        # mid tiles live through P4 only
        mid_ctx = tc.tile_pool(name="mid", bufs=1)
        mid = mid_ctx.__enter__()
        cosq = mid.tile([128, NT, S], BF16, tag="cosq")
        nc.sync.dma_start(out=cosq, in_=r3(cosq_d.ap()))
        sinq = mid.tile([128, NT, S], BF16, tag="sinq")
        nc.sync.dma_start(out=sinq, in_=r3(sinq_d.ap()))
        if share_qk:
            cosk, sink = cosq, sinq
        else:
            cosk = mid.tile([128, NT, S], BF16, tag="cosk")
            nc.sync.dma_start(out=cosk, in_=r3(cosk_d.ap()))
            sink = mid.tile([128, NT, S], BF16, tag="sink")
            nc.sync.dma_start(out=sink, in_=r3(sink_d.ap()))
        vsb8 = mid.tile([128, NT, E], FP8, tag="vsb8")   # v in fp8  [s, e]
        qr = mid.tile([128, NT, S], BF16, tag="qr")      # rope(q)^T
        kr = mid.tile([128, NT, S], BF16, tag="kr")      # rope(k)^T

        # prefetch the rest of the weights (slots free up as phases finish)
        wo_sb = load_w8("Wo")
        wxr_sb = load_wb("Wxr")
        wyr_sb = load_w8("Wyr")
        wxz_sb = load_wb("Wxz")
        wyz_sb = load_w8("Wyz")
        wxg_sb = load_wb("Wxg")
        wyg_sb = load_w8("Wyg")

        # =========== P2: v = (x @ Wv) + bv  (seq-major, fp8 out) ============
        for st in range(NT):
            ss = slice(st * 128, (st + 1) * 128)
            for c in range(2):
                sl = slice(c * 512, (c + 1) * 512)
                ps = psum.tile([128, 512], F32, tag="mm", bufs=2, name="ps_v")
                for kp in range(NP):
                    nc.tensor.matmul(
                        ps, lhsT=xT8[:, 2 * kp:2 * kp + 2, ss],
                        rhs=wv_sb[:, 2 * kp:2 * kp + 2, sl],
                        start=(kp == 0), stop=False, perf_mode=DR)
                nc.tensor.matmul(ps, lhsT=ones1_b, rhs=bv_row[:, sl],
                                 start=False, stop=True)
                nc.vector.tensor_scalar(out=vsb8[:, st, sl], in0=ps,
                                        scalar1=1.0 / WS, scalar2=None,
                                        op0=ALU.mult)

        # =========== P3: q/k proj (fp8 DR) + RoPE (bf16) =====================
        with tc.tile_pool(name="p3", bufs=4) as p3:
            def proj_tile(t, w_sb, bias_sb):
                qs = p3.tile([128, S], BF16, tag="qs")
                for c in range(2):
                    sl = slice(c * 512, (c + 1) * 512)
                    ps = psum.tile([128, 512], F32, tag="mm", bufs=2, name="ps_qk")
                    for kp in range(NP):
                        nc.tensor.matmul(
                            ps, lhsT=w_sb[:, 2 * kp:2 * kp + 2, t * 128:(t + 1) * 128],
                            rhs=xT8[:, 2 * kp:2 * kp + 2, sl],
                            start=(kp == 0), stop=(kp == NP - 1), perf_mode=DR)
                    nc.scalar.activation(qs[:, sl], ps, AF.Identity,
                                         bias=bias_sb[:, t:t + 1], scale=1.0 / WS)
                return qs

            def rotate(t, qs, cos_t, sin_t, dst):
                # dst = qs*cos + swap_pairs(qs)*sin' with the pair-swap done
                # by a negative-stride AP and the +- signs baked into sin'.
                for c in range(2):
                    sl = slice(c * 512, (c + 1) * 512)
                    t1 = p3.tile([128, 512], BF16, tag="t1")
                    nc.vector.tensor_mul(t1, qs[:, sl], cos_t[:, t, sl])
                    qsw = qs[:, sl].rearrange("p (a b) -> p a b", b=2)[:, :, ::-1]
                    t2 = p3.tile([128, 512], BF16, tag="t2")
                    nc.vector.tensor_mul(t2, qsw, sin_t[:, t, sl])
                    nc.vector.tensor_add(dst[:, t, sl], t1, t2)

            for t in range(NT):
                qs = proj_tile(t, wq_sb, bq_sb)
                rotate(t, qs, cosq, sinq, qr)
                ks = proj_tile(t, wk_sb, bk_sb)
                rotate(t, ks, cosk, sink, kr)

        # =========== P4: attention per head ==================================
        with tc.tile_pool(name="p4", bufs=2) as p4:
            ctx8 = res.tile([128, NT, S], FP8, tag="ctx8")  # 16*ctx in fp8
            for h in range(H):
                expT = p4.tile([128, NT, S], FP8, tag="expT")
                for jt in range(NT):
                    i0 = jt * 128
                    ps = psum.tile([128, 1024], F32, tag="sc", bufs=2, name="ps_sc")
                    if i0 < 512:
                        nc.tensor.matmul(
                            ps[:, i0:512],
                            lhsT=kr[:, h, i0:i0 + 128],
                            rhs=qr[:, h, i0:512], start=True, stop=True)
                        nc.tensor.matmul(
                            ps[:, 512:1024],
                            lhsT=kr[:, h, i0:i0 + 128],
                            rhs=qr[:, h, 512:1024], start=True, stop=True)
                    else:
                        nc.tensor.matmul(
                            ps[:, i0:1024],
                            lhsT=kr[:, h, i0:i0 + 128],
                            rhs=qr[:, h, i0:1024], start=True, stop=True)
                    nc.scalar.activation(expT[:, jt, i0:1024], ps[:, i0:1024],
                                         AF.Exp, scale=SCALE)
                    # causal mask on the diagonal 128x128 block
                    nc.gpsimd.affine_select(
                        out=expT[:, jt, i0:i0 + 128], in_=expT[:, jt, i0:i0 + 128],
                        pattern=[[1, 128]], compare_op=ALU.is_ge,
                        fill=0.0, base=0, channel_multiplier=-1)
                    # zero the strip the DoubleRow pair-partner reads above
                    # the diagonal (odd tiles, queries [i0-128, i0))
                    if jt % 2 == 1:
                        nc.vector.memset(expT[:, jt, i0 - 128:i0], 0.0)
                # den broadcast to all partitions via ones(1/CS) lhsT, then
                # reciprocal directly yields the CS/den normalizer per query.
                for c in range(2):
                    cs, ce = c * 512, (c + 1) * 512
                    jps = [jp for jp in range(NP) if jp * 256 < ce]
                    dps = psum.tile([128, 512], F32, tag="dc", bufs=2, name="ps_den")
                    for n, jp in enumerate(jps):
                        a = max(jp * 256, cs)
                        nc.tensor.matmul(
                            dps[:, a - cs:512], lhsT=ones8,
                            rhs=expT[:, 2 * jp:2 * jp + 2, a:ce],
                            start=(n == 0), stop=(n == len(jps) - 1), perf_mode=DR)
                    rf = p4.tile([128, 512], F32, tag="rf")
                    nc.vector.reciprocal_approx_fast(out=rf, in_=dps)
                    cps = psum.tile([128, 512], F32, tag="dc", bufs=2, name="ps_ctx")
                    for n, jp in enumerate(jps):
                        a = max(jp * 256, cs)
                        nc.tensor.matmul(
                            cps[:, a - cs:512],
                            lhsT=vsb8[:, 2 * jp:2 * jp + 2, h * 128:(h + 1) * 128],
                            rhs=expT[:, 2 * jp:2 * jp + 2, a:ce],
                            start=(n == 0), stop=(n == len(jps) - 1), perf_mode=DR)
                    nc.vector.tensor_mul(ctx8[:, h, cs:ce], cps, rf)

        # =========== P5: y8 = 8*relu(ctx Wo + bo)  (feature-major) ==========
        mid_ctx.__exit__(None, None, None)
        res2 = top.enter_context(tc.tile_pool(name="res2", bufs=1))
        yT8 = res2.tile([128, NT, S], FP8, tag="yT8")    # 8*y in fp8
        rx = res2.tile([128, NT, S], BF16, tag="rx")     # (r*x)^T bf16
        for t in range(NT):
            for c in range(2):
                sl = slice(c * 512, (c + 1) * 512)
                ps = psum.tile([128, 512], F32, tag="mm", bufs=2, name="ps_y")
                for kp in range(NP):
                    nc.tensor.matmul(
                        ps, lhsT=wo_sb[:, 2 * kp:2 * kp + 2, t * 128:(t + 1) * 128],
                        rhs=ctx8[:, 2 * kp:2 * kp + 2, sl],
                        start=(kp == 0), stop=(kp == NP - 1), perf_mode=DR)
                # psum = WS*CS*(ctx@Wo); y8 = relu(psum*YS/(WS*CS) + YS*bo)
                nc.scalar.activation(yT8[:, t, sl], ps, AF.Relu,
                                     bias=bo8_sb[:, t:t + 1],
                                     scale=YS / (WS * CS))

        # ===== P6: r = sigmoid(x Wxr + y Wyr); rx = r * xT (bf16) ============
        with tc.tile_pool(name="p6", bufs=3) as p6:
            for t in range(NT):
                for c in range(2):
                    sl = slice(c * 512, (c + 1) * 512)
                    ps = psum.tile([128, 512], F32, tag="mm", bufs=2, name="ps_r")
                    for kt in range(NT):
                        nc.tensor.matmul(
                            ps, lhsT=wxr_sb[:, kt, t * 128:(t + 1) * 128],
                            rhs=xTb[:, kt, sl], start=(kt == 0), stop=False)
                    for kp in range(NP):
                        nc.tensor.matmul(
                            ps, lhsT=wyr_sb[:, 2 * kp:2 * kp + 2, t * 128:(t + 1) * 128],
                            rhs=yT8[:, 2 * kp:2 * kp + 2, sl],
                            start=False, stop=(kp == NP - 1), perf_mode=DR)
                    rt = p6.tile([128, 512], BF16, tag="rt")
                    nc.scalar.activation(rt, ps, AF.Sigmoid, scale=1.0 / WS)
                    nc.vector.tensor_mul(rx[:, t, sl], rt, xTb[:, t, sl])

        # =========== P7: z/h + gated combine (seq-major, single pass) ========
        with tc.tile_pool(name="p7", bufs=2) as p7:
            for st in range(NT):
                ss = slice(st * 128, (st + 1) * 128)
                xf = p7.tile([128, E], F32, tag="xf")
                nc.sync.dma_start(out=xf, in_=xb_d.ap()[ss, :])
                ot = p7.tile([128, E], F32, tag="ot")
                for c in range(2):
                    sl = slice(c * 512, (c + 1) * 512)
                    zps = psum.tile([128, 512], F32, tag="mm", bufs=2, name="ps_z")
                    for kt in range(NT):
                        nc.tensor.matmul(zps, lhsT=xTb[:, kt, ss],
                                         rhs=wxz_sb[:, kt, sl],
                                         start=(kt == 0), stop=False)
                    for kp in range(NP):
                        nc.tensor.matmul(zps, lhsT=yT8[:, 2 * kp:2 * kp + 2, ss],
                                         rhs=wyz_sb[:, 2 * kp:2 * kp + 2, sl],
                                         start=False, stop=False, perf_mode=DR)
                    nc.tensor.matmul(zps, lhsT=ones1_b, rhs=bxz_row[:, sl],
                                     start=False, stop=True)
                    zt = p7.tile([128, 512], F32, tag="zt")
                    nc.scalar.activation(zt, zps, AF.Sigmoid, scale=1.0 / WS)
                    hps = psum.tile([128, 512], F32, tag="mm", bufs=2, name="ps_h")
                    for kt in range(NT):
                        nc.tensor.matmul(hps, lhsT=rx[:, kt, ss],
                                         rhs=wxg_sb[:, kt, sl],
                                         start=(kt == 0), stop=False)
                    for kp in range(NP):
                        nc.tensor.matmul(hps, lhsT=yT8[:, 2 * kp:2 * kp + 2, ss],
                                         rhs=wyg_sb[:, 2 * kp:2 * kp + 2, sl],
                                         start=False, stop=(kp == NP - 1), perf_mode=DR)
                    ht = p7.tile([128, 512], F32, tag="ht")
                    nc.scalar.activation(ht, hps, AF.Tanh, scale=1.0 / WS)
                    dt = p7.tile([128, 512], F32, tag="dt")
                    nc.gpsimd.tensor_sub(dt, ht, xf[:, sl])
                    zd = p7.tile([128, 512], F32, tag="zd")
                    nc.vector.tensor_mul(zd, zt, dt)
                    nc.vector.tensor_add(ot[:, sl], xf[:, sl], zd)
                nc.sync.dma_start(out=out_d.ap()[ss, :], in_=ot)

    nc.compile()
    return nc


# ---------------- host-side packing -----------------------------------------

def _pack_w(w, scale, npdt):
    return np.ascontiguousarray(
        (np.asarray(w, np.float32) * scale).astype(npdt)
        .reshape(NT, 128, E).transpose(1, 0, 2).reshape(128, NT * E))


def _pack_fm(m, npdt):
    # [E, S]-logical feature-major -> [128, NT*S]
    return np.ascontiguousarray(
        m.astype(npdt).reshape(NT, 128, S).transpose(1, 0, 2).reshape(128, NT * S))


def _pack_bias_fm(b, scale=1.0):
    return np.ascontiguousarray(
        (np.asarray(b, np.float32) * scale).reshape(NT, 128).T)


_INV = None


def _inv_pair():
    global _INV
    if _INV is None:
        inv = 1.0 / (10000.0 ** (np.arange(0, E, 2, dtype=np.float32) / np.float32(E)))
        _INV = np.repeat(inv.astype(np.float64), 2)  # pair-expanded [E]
    return _INV


def _tables(idx):
    f = _inv_pair()[:, None] * idx.astype(np.float64)[None, :]  # [E, S]
    sn = np.sin(f).astype(np.float32)
    sn[0::2, :] *= -1.0  # sign baked in for the pair-swap AP read
    return (_pack_fm(np.cos(f).astype(np.float32), NPBF16),
            _pack_fm(sn, NPBF16))


def make_in_maps(inputs, share_qk):
    x = np.asarray(inputs["x"], dtype=np.float32)
    qi = np.asarray(inputs["query_index"])
    ki = np.asarray(inputs["key_index"])
    common = {
        "bq": _pack_bias_fm(np.asarray(inputs["bq"])),
        "bk": _pack_bias_fm(np.asarray(inputs["bk"])),
        "bo8": _pack_bias_fm(np.asarray(inputs["bo"]), YS),
        "bv_row": (np.asarray(inputs["bv"], np.float32) * WS).astype(NPBF16).reshape(1, E),
        "bxz_row": (np.asarray(inputs["bxz"], np.float32) * WS).astype(NPBF16).reshape(1, E),
        "ones8": np.full((128, 256), 1.0 / CS, NPFP8),
    }
    for nm in ("Wq", "Wk", "Wv", "Wo"):
        common[nm] = _pack_w(inputs[nm], WS, NPFP8)
    for nm in ("Wyr", "Wyz", "Wyg"):
        common[nm] = _pack_w(inputs[nm], WS / YS, NPFP8)
    for nm in ("Wxr", "Wxz", "Wxg"):
        common[nm] = _pack_w(inputs[nm], WS, NPBF16)
    in_maps = []
    for b in range(B):
        m = dict(common)
        xb = np.ascontiguousarray(x[b])
        m["xb"] = xb
        xt = xb.T  # [E, S]
        m["xtb"] = _pack_fm(xt, NPBF16)
        m["xt8"] = _pack_fm(xt, NPFP8)
        m["cosq"], m["sinq"] = _tables(qi[b])
        if not share_qk:
            m["cosk"], m["sink"] = _tables(ki[b])
        in_maps.append(m)
    return in_maps


def kernel(**inputs):
    qi = np.asarray(inputs["query_index"])
    ki = np.asarray(inputs["key_index"])
    share_qk = bool(np.array_equal(qi, ki))

    key = ("k", share_qk)
    if key not in _COMPILED:
        _COMPILED[key] = _build(share_qk)
    nc = _COMPILED[key]

    in_maps = make_in_maps(inputs, share_qk)
    global _dbg_in_maps
    _dbg_in_maps = in_maps
    res = bass_utils.run_bass_kernel_spmd(nc, in_maps, core_ids=list(range(NC)))
    out = np.stack([res.results[b]["out"] for b in range(B)]).astype(np.float32)
    return out


# revision 38
# speedup vs baseline: 1.1316x; 1.1316x over previous
# Trainium2 Bass kernel for nn_EpisodeMultiheadAttentionBlock.
# B=8, S=1024, E=1024, H=8 heads, HD=128. Data-parallel over batch: core b
# computes batch element b. Self-contained: only needs /opt/trn_rl_repo on path.
#
# v3: fp8(e4m3) DoubleRow matmuls for QKV/Wo/y-side gates and attention AV/den
# (weights pre-scaled by powers of 2, dequantized via activation scales);
# host-precomputed x^T and RoPE cos/sin tables; bf16 where precision demands
# it (scores, x-side gate matmuls, rope rotation). Single long-lived PSUM
# pool (no phase barriers), DMA loads ordered for prefetch, elementwise work
# spread across DVE/Act/Pool.
import sys
import numpy as np

sys.path.insert(0, "/opt/trn_rl_repo")

import ml_dtypes  # noqa: E402
import concourse.bass as bass  # noqa: E402
import concourse.mybir as mybir  # noqa: E402
import concourse.tile as tile  # noqa: E402
from concourse import bacc  # noqa: E402
from concourse import bass_utils  # noqa: E402

B, S, E, H = 8, 1024, 1024, 8
HD = E // H  # 128
NT = E // 128  # 8 e-tiles / s-tiles
NP = NT // 2  # 4 DoubleRow k-tile pairs
NC = 8  # cores
BF16 = mybir.dt.bfloat16
F32 = mybir.dt.float32
FP8 = mybir.dt.float8e4
AF = mybir.ActivationFunctionType
DR = mybir.MatmulPerfMode.DoubleRow
ALU = mybir.AluOpType
NPBF16 = ml_dtypes.bfloat16
NPFP8 = ml_dtypes.float8_e4m3

WS = 32.0  # weight pre-scale for fp8/bf16 weights
YS = 8.0   # y stored as 8*y in fp8
CS = 16.0  # ctx stored as 16*ctx in fp8

_COMPILED = {}


def _build(share_qk: bool):
    nc = bacc.Bacc("TRN2", target_bir_lowering=False, debug=False, num_devices=NC)

    # ---- DRAM tensors -------------------------------------------------------
    xb_d = nc.dram_tensor("xb", [S, E], F32, kind="ExternalInput")
    xtb_d = nc.dram_tensor("xtb", [128, NT * S], BF16, kind="ExternalInput")
    xt8_d = nc.dram_tensor("xt8", [128, NT * S], FP8, kind="ExternalInput")
    w8_d = {
        nm: nc.dram_tensor(nm, [128, NT * E], FP8, kind="ExternalInput")
        for nm in ("Wq", "Wk", "Wv", "Wo", "Wxr", "Wyr", "Wyz", "Wyg")
    }
    wb_d = {
        nm: nc.dram_tensor(nm, [128, NT * E], BF16, kind="ExternalInput")
        for nm in ("Wxz", "Wxg")
    }
    bq_d = nc.dram_tensor("bq", [128, NT], F32, kind="ExternalInput")
    bk_d = nc.dram_tensor("bk", [128, NT], F32, kind="ExternalInput")
    bo8_d = nc.dram_tensor("bo8", [128, NT], F32, kind="ExternalInput")
    bv_row_d = nc.dram_tensor("bv_row", [1, E], BF16, kind="ExternalInput")   # 32*bv
    bxz_row_d = nc.dram_tensor("bxz_row", [1, E], BF16, kind="ExternalInput")  # 32*bxz
    cosq_d = nc.dram_tensor("cosq", [128, NT * S], BF16, kind="ExternalInput")
    sinq_d = nc.dram_tensor("sinq", [128, NT * S], BF16, kind="ExternalInput")
    if not share_qk:
        cosk_d = nc.dram_tensor("cosk", [128, NT * S], BF16, kind="ExternalInput")
        sink_d = nc.dram_tensor("sink", [128, NT * S], BF16, kind="ExternalInput")
    pmat_d = nc.dram_tensor("pmat", [128, 128], BF16, kind="ExternalInput")
    ones8_d = nc.dram_tensor("ones8", [128, 256], FP8, kind="ExternalInput")
    out_d = nc.dram_tensor("out", [S, E], F32, kind="ExternalOutput")

    SCALE = 1.0 / float(np.sqrt(HD))

    def r3(ap):
        return ap.rearrange("p (t s) -> p t s", t=NT)

    with tile.TileContext(nc) as tc:
      from contextlib import ExitStack

      with ExitStack() as top:
        res = top.enter_context(tc.tile_pool(name="res", bufs=1))
        consts = top.enter_context(tc.tile_pool(name="consts", bufs=1))
        wp8 = top.enter_context(tc.tile_pool(name="wp8", bufs=4))
        wpb = top.enter_context(tc.tile_pool(name="wpb", bufs=2))
        psum = top.enter_context(tc.tile_pool(name="psum", bufs=1, space="PSUM"))

        def load_w8(nm):
            t = wp8.tile([128, NT, E], FP8, tag="W8", name=f"w_{nm}")
            nc.sync.dma_start(out=t, in_=w8_d[nm].ap().rearrange("p (t e) -> p t e", t=NT))
            return t

        def load_wb(nm, tag="Wb"):
            t = wpb.tile([128, NT, E], BF16, tag=tag, name=f"w_{nm}", bufs=(1 if tag == "Wb2" else None))
            nc.sync.dma_start(out=t, in_=wb_d[nm].ap().rearrange("p (t e) -> p t e", t=NT))
            return t

        # ------- loads in prefetch order (tiny consts first: the P2 bias
        # matmul sits early in PE program order, so bv_row must land first
        # or it head-of-line-blocks the whole PE queue) -------------------
        pmat = consts.tile([128, 128], BF16, tag="pmat")
        nc.sync.dma_start(out=pmat, in_=pmat_d.ap())
        bq_sb = consts.tile([128, NT], F32, tag="bq")
        nc.sync.dma_start(out=bq_sb, in_=bq_d.ap())
        bk_sb = consts.tile([128, NT], F32, tag="bk")
        nc.sync.dma_start(out=bk_sb, in_=bk_d.ap())
        bo8_sb = consts.tile([128, NT], F32, tag="bo8")
        nc.sync.dma_start(out=bo8_sb, in_=bo8_d.ap())
        bv_row = consts.tile([1, E], BF16, tag="bv_row")
        nc.sync.dma_start(out=bv_row, in_=bv_row_d.ap())
        bxz_row = consts.tile([1, E], BF16, tag="bxz_row")
        nc.sync.dma_start(out=bxz_row, in_=bxz_row_d.ap())
        ones8 = consts.tile([128, 2, 128], FP8, tag="ones8")  # value 1/CS
        nc.sync.dma_start(out=ones8, in_=ones8_d.ap().rearrange("p (a b) -> p a b", a=2))
        ones1_b = consts.tile([1, 128], BF16, tag="ones1_b")
        nc.vector.memset(ones1_b, 1.0)

        xT8 = res.tile([128, NT, S], FP8, tag="xT8")
        xt8_r = r3(xt8_d.ap())
        nc.sync.dma_start(out=xT8[:, 0:2, :], in_=xt8_r[:, 0:2, :])
        wv_sb = wp8.tile([128, NT, E], FP8, tag="W8", name="w_Wv")
        wv_r = w8_d["Wv"].ap().rearrange("p (t e) -> p t e", t=NT)
        nc.sync.dma_start(out=wv_sb[:, :, 0:512], in_=wv_r[:, :, 0:512])
        nc.sync.dma_start(out=xT8[:, 2:NT, :], in_=xt8_r[:, 2:NT, :])
        nc.sync.dma_start(out=wv_sb[:, :, 512:E], in_=wv_r[:, :, 512:E])
        wq_sb = load_w8("Wq")
        wk_sb = load_w8("Wk")

        # mid tiles live through P4 only
        mid_ctx = tc.tile_pool(name="mid", bufs=1)
        mid = mid_ctx.__enter__()
        vsb8 = mid.tile([128, NT, E], FP8, tag="vsb8")   # v in fp8  [s, e]
        qr = mid.tile([128, NT, S], BF16, tag="qr")      # rope(q)^T
        kr = mid.tile([128, NT, S], BF16, tag="kr")      # rope(k)^T
        # rope tables live through P3 only
        tab_ctx = tc.tile_pool(name="tab", bufs=1)
        tab = tab_ctx.__enter__()
        cosq = tab.tile([128, NT, S], BF16, tag="cosq")
        nc.sync.dma_start(out=cosq, in_=r3(cosq_d.ap()))
        sinq = tab.tile([128, NT, S], BF16, tag="sinq")
        nc.sync.dma_start(out=sinq, in_=r3(sinq_d.ap()))

        xTb = res.tile([128, NT, S], BF16, tag="xTb")
        nc.sync.dma_start(out=xTb, in_=r3(xtb_d.ap()))
        # prefetch the rest of the weights (slots free up as phases finish)
        wo_sb = load_w8("Wo")
        wxr_sb = load_w8("Wxr")
        wyr_sb = load_w8("Wyr")
        wxz_sb = load_wb("Wxz")
        wyz_sb = load_w8("Wyz")
        wxg_sb = load_wb("Wxg", tag="Wb2")
        wyg_sb = load_w8("Wyg")

        # =========== P2: v = (x @ Wv) + bv  (seq-major, fp8 out) ============
        for st in range(NT):
            ss = slice(st * 128, (st + 1) * 128)
            for c in range(2):
                sl = slice(c * 512, (c + 1) * 512)
                ps = psum.tile([128, 512], F32, tag="mm", bufs=2, name="ps_v")
                for kp in range(NP):
                    nc.tensor.matmul(
                        ps, lhsT=xT8[:, 2 * kp:2 * kp + 2, ss],
                        rhs=wv_sb[:, 2 * kp:2 * kp + 2, sl],
                        start=(kp == 0), stop=False, perf_mode=DR)
                nc.tensor.matmul(ps, lhsT=ones1_b, rhs=bv_row[:, sl],
                                 start=False, stop=True)
                nc.vector.tensor_scalar(out=vsb8[:, st, sl], in0=ps,
                                        scalar1=1.0 / WS, scalar2=None, op0=ALU.mult)

        # =========== P3: q/k proj (fp8 DR) + RoPE (bf16) =====================
        with tc.tile_pool(name="p3", bufs=4) as p3:
            def proj_tile(t, w_sb, bias_sb):
                qs = p3.tile([128, S], BF16, tag="qs")
                for c in range(2):
                    sl = slice(c * 512, (c + 1) * 512)
                    ps = psum.tile([128, 512], F32, tag="mm", bufs=2, name="ps_qk")
                    for kp in range(NP):
                        nc.tensor.matmul(
                            ps, lhsT=w_sb[:, 2 * kp:2 * kp + 2, t * 128:(t + 1) * 128],
                            rhs=xT8[:, 2 * kp:2 * kp + 2, sl],
                            start=(kp == 0), stop=(kp == NP - 1), perf_mode=DR)
                    nc.scalar.activation(qs[:, sl], ps, AF.Identity,
                                         bias=bias_sb[:, t:t + 1], scale=1.0 / WS)
                return qs

            def rotate(t, qs, cos_t, sin_t, dst):
                for c in range(2):
                    sl = slice(c * 512, (c + 1) * 512)
                    sh = psum.tile([128, 512], F32, tag="mm", bufs=2, name="ps_sh")
                    nc.tensor.matmul(sh, lhsT=pmat, rhs=qs[:, sl], start=True, stop=True)
                    shb = p3.tile([128, 512], BF16, tag="shb", bufs=2)
                    nc.scalar.copy(shb, sh)
                    t1 = p3.tile([128, 512], BF16, tag="t1", bufs=3)
                    nc.vector.tensor_mul(t1, qs[:, sl], cos_t[:, t, sl])
                    t2 = p3.tile([128, 512], BF16, tag="t2", bufs=3)
                    nc.vector.tensor_mul(t2, shb, sin_t[:, t, sl])
                    nc.gpsimd.tensor_add(dst[:, t, sl], t1, t2)

            if share_qk:
                prev = None
                for t in range(NT):
                    qs = proj_tile(t, wq_sb, bq_sb)
                    ks = proj_tile(t, wk_sb, bk_sb)
                    if prev is not None:
                        pt, pq, pk = prev
                        rotate(pt, pq, cosq, sinq, qr)
                        rotate(pt, pk, cosq, sinq, kr)
                    prev = (t, qs, ks)
                pt, pq, pk = prev
                rotate(pt, pq, cosq, sinq, qr)
                rotate(pt, pk, cosq, sinq, kr)
            else:
                # correctness-first fallback: q pass, reload tables, k pass
                for t in range(NT):
                    qs = proj_tile(t, wq_sb, bq_sb)
                    rotate(t, qs, cosq, sinq, qr)
                cosk = tab.tile([128, NT, S], BF16, tag="cosq")
                nc.sync.dma_start(out=cosk, in_=r3(cosk_d.ap()))
                sink = tab.tile([128, NT, S], BF16, tag="sinq")
                nc.sync.dma_start(out=sink, in_=r3(sink_d.ap()))
                for t in range(NT):
                    ks = proj_tile(t, wk_sb, bk_sb)
                    rotate(t, ks, cosk, sink, kr)

        # =========== P4: attention per head ==================================
        tab_ctx.__exit__(None, None, None)
        with tc.tile_pool(name="p4", bufs=2) as p4:
            ctx8 = res.tile([128, NT, S], FP8, tag="ctx8")  # 16*ctx in fp8

            def emit_scores(h):
                expT = p4.tile([128, NT, S], FP8, tag="expT", bufs=3)
                for jt in range(NT):
                    i0 = jt * 128
                    ps = psum.tile([128, 1024], F32, tag="sc", bufs=2, name="ps_sc")
                    if i0 < 512:
                        nc.tensor.matmul(
                            ps[:, i0:512],
                            lhsT=kr[:, h, i0:i0 + 128],
                            rhs=qr[:, h, i0:512], start=True, stop=True)
                        nc.tensor.matmul(
                            ps[:, 512:1024],
                            lhsT=kr[:, h, i0:i0 + 128],
                            rhs=qr[:, h, 512:1024], start=True, stop=True)
                    else:
                        nc.tensor.matmul(
                            ps[:, i0:1024],
                            lhsT=kr[:, h, i0:i0 + 128],
                            rhs=qr[:, h, i0:1024], start=True, stop=True)
                    nc.scalar.activation(expT[:, jt, i0:1024], ps[:, i0:1024],
                                         AF.Exp, scale=SCALE)
                    nc.gpsimd.affine_select(
                        out=expT[:, jt, i0:i0 + 128], in_=expT[:, jt, i0:i0 + 128],
                        pattern=[[1, 128]], compare_op=ALU.is_ge,
                        fill=0.0, base=0, channel_multiplier=-1)
                    if jt % 2 == 1:
                        nc.vector.memset(expT[:, jt, i0 - 128:i0], 0.0)
                return expT

            def emit_av(h, expT):
                for c in range(2):
                    cs, ce = c * 512, (c + 1) * 512
                    jps = [jp for jp in range(NP) if jp * 256 < ce]
                    dps = psum.tile([128, 512], F32, tag="dc", bufs=2, name="ps_den")
                    for n, jp in enumerate(jps):
                        a = max(jp * 256, cs)
                        nc.tensor.matmul(
                            dps[:, a - cs:512], lhsT=ones8,
                            rhs=expT[:, 2 * jp:2 * jp + 2, a:ce],
                            start=(n == 0), stop=(n == len(jps) - 1), perf_mode=DR)
                    rf = p4.tile([128, 512], F32, tag="rf")
                    nc.vector.reciprocal_approx_fast(out=rf, in_=dps)
                    cps = psum.tile([128, 512], F32, tag="dc", bufs=2, name="ps_ctx")
                    for n, jp in enumerate(jps):
                        a = max(jp * 256, cs)
                        nc.tensor.matmul(
                            cps[:, a - cs:512],
                            lhsT=vsb8[:, 2 * jp:2 * jp + 2, h * 128:(h + 1) * 128],
                            rhs=expT[:, 2 * jp:2 * jp + 2, a:ce],
                            start=(n == 0), stop=(n == len(jps) - 1), perf_mode=DR)
                    nc.vector.tensor_mul(ctx8[:, h, cs:ce], cps, rf)

            prev_h = None
            for h in range(H):
                expT = emit_scores(h)
                if prev_h is not None:
                    emit_av(prev_h[0], prev_h[1])
                prev_h = (h, expT)
            emit_av(prev_h[0], prev_h[1])

        # =========== P5: y8 = 8*relu(ctx Wo + bo)  (feature-major) ==========
        mid_ctx.__exit__(None, None, None)
        res2 = top.enter_context(tc.tile_pool(name="res2", bufs=1))
        yT8 = res2.tile([128, NT, S], FP8, tag="yT8")    # 8*y in fp8
        rx = res2.tile([128, NT, S], BF16, tag="rx")     # (r*x)^T bf16
        for t in range(NT):
            for c in range(2):
                sl = slice(c * 512, (c + 1) * 512)
                ps = psum.tile([128, 512], F32, tag="mm", bufs=2, name="ps_y")
                for kp in range(NP):
                    nc.tensor.matmul(
                        ps, lhsT=wo_sb[:, 2 * kp:2 * kp + 2, t * 128:(t + 1) * 128],
                        rhs=ctx8[:, 2 * kp:2 * kp + 2, sl],
                        start=(kp == 0), stop=(kp == NP - 1), perf_mode=DR)
                # psum = WS*CS*(ctx@Wo); y8 = relu(psum*YS/(WS*CS) + YS*bo)
                nc.scalar.activation(yT8[:, t, sl], ps, AF.Relu,
                                     bias=bo8_sb[:, t:t + 1],
                                     scale=YS / (WS * CS))

        # ===== P6: r = sigmoid(x Wxr + y Wyr); rx = r * xT (bf16) ============
        with tc.tile_pool(name="p6", bufs=3) as p6:
            for t in range(NT):
                for c in range(2):
                    sl = slice(c * 512, (c + 1) * 512)
                    ps = psum.tile([128, 512], F32, tag="mm", bufs=2, name="ps_r")
                    for kp in range(NP):
                        nc.tensor.matmul(
                            ps, lhsT=wxr_sb[:, 2 * kp:2 * kp + 2, t * 128:(t + 1) * 128],
                            rhs=xT8[:, 2 * kp:2 * kp + 2, sl],
                            start=(kp == 0), stop=False, perf_mode=DR)
                    for kp in range(NP):
                        nc.tensor.matmul(
                            ps, lhsT=wyr_sb[:, 2 * kp:2 * kp + 2, t * 128:(t + 1) * 128],
                            rhs=yT8[:, 2 * kp:2 * kp + 2, sl],
                            start=False, stop=(kp == NP - 1), perf_mode=DR)
                    rt = p6.tile([128, 512], BF16, tag="rt")
                    nc.scalar.activation(rt, ps, AF.Sigmoid, scale=1.0 / WS)
                    nc.vector.tensor_mul(rx[:, t, sl], rt, xTb[:, t, sl])

        # =========== P7: z/h + gated combine (seq-major, single pass) ========
        with tc.tile_pool(name="p7", bufs=2) as p7:
            for st in range(NT):
                ss = slice(st * 128, (st + 1) * 128)
                xf = p7.tile([128, E], F32, tag="xf")
                nc.sync.dma_start(out=xf, in_=xb_d.ap()[ss, :])
                ot = p7.tile([128, E], F32, tag="ot")
                for c in range(2):
                    sl = slice(c * 512, (c + 1) * 512)
                    zps = psum.tile([128, 512], F32, tag="mm", bufs=2, name="ps_z")
                    for kt in range(NT):
                        nc.tensor.matmul(zps, lhsT=xTb[:, kt, ss],
                                         rhs=wxz_sb[:, kt, sl],
                                         start=(kt == 0), stop=False)
                    for kp in range(NP):
                        nc.tensor.matmul(zps, lhsT=yT8[:, 2 * kp:2 * kp + 2, ss],
                                         rhs=wyz_sb[:, 2 * kp:2 * kp + 2, sl],
                                         start=False, stop=False, perf_mode=DR)
                    nc.tensor.matmul(zps, lhsT=ones1_b, rhs=bxz_row[:, sl],
                                     start=False, stop=True)
                    zt = p7.tile([128, 512], F32, tag="zt")
                    nc.scalar.activation(zt, zps, AF.Sigmoid, scale=1.0 / WS)
                    hps = psum.tile([128, 512], F32, tag="mm", bufs=2, name="ps_h")
                    for kt in range(NT):
                        nc.tensor.matmul(hps, lhsT=rx[:, kt, ss],
                                         rhs=wxg_sb[:, kt, sl],
                                         start=(kt == 0), stop=False)
                    for kp in range(NP):
                        nc.tensor.matmul(hps, lhsT=yT8[:, 2 * kp:2 * kp + 2, ss],
                                         rhs=wyg_sb[:, 2 * kp:2 * kp + 2, sl],
                                         start=False, stop=(kp == NP - 1), perf_mode=DR)
                    ht = p7.tile([128, 512], F32, tag="ht")
                    nc.scalar.activation(ht, hps, AF.Tanh, scale=1.0 / WS)
                    dt = p7.tile([128, 512], F32, tag="dt")
                    nc.vector.tensor_sub(dt, ht, xf[:, sl])
                    zd = p7.tile([128, 512], F32, tag="zd")
                    nc.vector.tensor_mul(zd, zt, dt)
                    nc.vector.tensor_add(ot[:, sl], xf[:, sl], zd)
                nc.sync.dma_start(out=out_d.ap()[ss, :], in_=ot)

    nc.compile()
    return nc


# ---------------- host-side packing -----------------------------------------

def _pack_w(w, scale, npdt):
    return np.ascontiguousarray(
        (np.asarray(w, np.float32) * scale).astype(npdt)
        .reshape(NT, 128, E).transpose(1, 0, 2).reshape(128, NT * E))


def _pack_fm(m, npdt):
    # [E, S]-logical feature-major -> [128, NT*S]
    return np.ascontiguousarray(
        m.astype(npdt).reshape(NT, 128, S).transpose(1, 0, 2).reshape(128, NT * S))


def _pack_bias_fm(b, scale=1.0):
    return np.ascontiguousarray(
        (np.asarray(b, np.float32) * scale).reshape(NT, 128).T)


_INV = None


def _inv_pair():
    global _INV
    if _INV is None:
        inv = 1.0 / (10000.0 ** (np.arange(0, E, 2, dtype=np.float32) / np.float32(E)))
        _INV = np.repeat(inv.astype(np.float64), 2)  # pair-expanded [E]
    return _INV


def _tables(idx):
    f = _inv_pair()[:, None] * idx.astype(np.float64)[None, :]  # [E, S]
    return (_pack_fm(np.cos(f).astype(np.float32), NPBF16),
            _pack_fm(np.sin(f).astype(np.float32), NPBF16))


def _pmat():
    pm = np.zeros((128, 128), dtype=NPBF16)
    for i in range(64):
        pm[2 * i + 1, 2 * i] = -1.0
        pm[2 * i, 2 * i + 1] = 1.0
    return pm


def make_in_maps(inputs, share_qk):
    x = np.asarray(inputs["x"], dtype=np.float32)
    qi = np.asarray(inputs["query_index"])
    ki = np.asarray(inputs["key_index"])
    common = {
        "bq": _pack_bias_fm(np.asarray(inputs["bq"])),
        "bk": _pack_bias_fm(np.asarray(inputs["bk"])),
        "bo8": _pack_bias_fm(np.asarray(inputs["bo"]), YS),
        "bv_row": (np.asarray(inputs["bv"], np.float32) * WS).astype(NPBF16).reshape(1, E),
        "bxz_row": (np.asarray(inputs["bxz"], np.float32) * WS).astype(NPBF16).reshape(1, E),
        "ones8": np.full((128, 256), 1.0 / CS, NPFP8),
        "pmat": _pmat(),
    }
    for nm in ("Wq", "Wk", "Wv", "Wo", "Wxr"):
        common[nm] = _pack_w(inputs[nm], WS, NPFP8)
    for nm in ("Wyr", "Wyz", "Wyg"):
        common[nm] = _pack_w(inputs[nm], WS / YS, NPFP8)
    for nm in ("Wxz", "Wxg"):
        common[nm] = _pack_w(inputs[nm], WS, NPBF16)
    in_maps = []
    for b in range(B):
        m = dict(common)
        xb = np.ascontiguousarray(x[b])
        m["xb"] = xb
        xt = xb.T  # [E, S]
        m["xtb"] = _pack_fm(xt, NPBF16)
        m["xt8"] = _pack_fm(xt, NPFP8)
        m["cosq"], m["sinq"] = _tables(qi[b])
        if not share_qk:
            m["cosk"], m["sink"] = _tables(ki[b])
        in_maps.append(m)
    return in_maps


def kernel(**inputs):
    qi = np.asarray(inputs["query_index"])
    ki = np.asarray(inputs["key_index"])
    share_qk = bool(np.array_equal(qi, ki))

    key = ("k", share_qk)
    if key not in _COMPILED:
        _COMPILED[key] = _build(share_qk)
    nc = _COMPILED[key]

    in_maps = make_in_maps(inputs, share_qk)
    global _dbg_in_maps
    _dbg_in_maps = in_maps
    res = bass_utils.run_bass_kernel_spmd(nc, in_maps, core_ids=list(range(NC)))
    out = np.stack([res.results[b]["out"] for b in range(B)]).astype(np.float32)
    return out


# revision 42
# speedup vs baseline: 1.1613x; 1.0262x over previous
# Trainium2 Bass kernel for nn_EpisodeMultiheadAttentionBlock.
# B=8, S=1024, E=1024, H=8 heads, HD=128. Data-parallel over batch: core b
# computes batch element b. Self-contained: only needs /opt/trn_rl_repo on path.
#
# v3: fp8(e4m3) DoubleRow matmuls for QKV/Wo/y-side gates and attention AV/den
# (weights pre-scaled by powers of 2, dequantized via activation scales);
# host-precomputed x^T and RoPE cos/sin tables; bf16 where precision demands
# it (scores, x-side gate matmuls, rope rotation). Single long-lived PSUM
# pool (no phase barriers), DMA loads ordered for prefetch, elementwise work
# spread across DVE/Act/Pool.
import sys
import numpy as np

sys.path.insert(0, "/opt/trn_rl_repo")

import ml_dtypes  # noqa: E402
import concourse.bass as bass  # noqa: E402
import concourse.mybir as mybir  # noqa: E402
import concourse.tile as tile  # noqa: E402
from concourse import bacc  # noqa: E402
from concourse import bass_utils  # noqa: E402

B, S, E, H = 8, 1024, 1024, 8
HD = E // H  # 128
NT = E // 128  # 8 e-tiles / s-tiles
NP = NT // 2  # 4 DoubleRow k-tile pairs
NC = 8  # cores
BF16 = mybir.dt.bfloat16
F32 = mybir.dt.float32
FP8 = mybir.dt.float8e4
AF = mybir.ActivationFunctionType
DR = mybir.MatmulPerfMode.DoubleRow
ALU = mybir.AluOpType
NPBF16 = ml_dtypes.bfloat16
NPFP8 = ml_dtypes.float8_e4m3

WS = 32.0  # weight pre-scale for fp8/bf16 weights
YS = 8.0   # y stored as 8*y in fp8
CS = 16.0  # ctx stored as 16*ctx in fp8

_COMPILED = {}


def _build(share_qk: bool):
    nc = bacc.Bacc("TRN2", target_bir_lowering=False, debug=False, num_devices=NC)

    # ---- DRAM tensors -------------------------------------------------------
    xb_d = nc.dram_tensor("xb", [S, E], F32, kind="ExternalInput")
    xtb_d = nc.dram_tensor("xtb", [128, NT * S], BF16, kind="ExternalInput")
    xt8_d = nc.dram_tensor("xt8", [128, NT * S], FP8, kind="ExternalInput")
    w8_d = {
        nm: nc.dram_tensor(nm, [128, NT * E], FP8, kind="ExternalInput")
        for nm in ("Wq", "Wk", "Wv", "Wo", "Wxr", "Wyr", "Wyz", "Wyg")
    }
    wb_d = {
        nm: nc.dram_tensor(nm, [128, NT * E], BF16, kind="ExternalInput")
        for nm in ("Wxz", "Wxg")
    }
    bq_d = nc.dram_tensor("bq", [128, NT], F32, kind="ExternalInput")
    bk_d = nc.dram_tensor("bk", [128, NT], F32, kind="ExternalInput")
    bo8_d = nc.dram_tensor("bo8", [128, NT], F32, kind="ExternalInput")
    bv_row_d = nc.dram_tensor("bv_row", [1, E], BF16, kind="ExternalInput")   # 32*bv
    bxz_row_d = nc.dram_tensor("bxz_row", [1, E], BF16, kind="ExternalInput")  # 32*bxz
    cosq_d = nc.dram_tensor("cosq", [128, NT * S], BF16, kind="ExternalInput")
    sinq_d = nc.dram_tensor("sinq", [128, NT * S], BF16, kind="ExternalInput")
    if not share_qk:
        cosk_d = nc.dram_tensor("cosk", [128, NT * S], BF16, kind="ExternalInput")
        sink_d = nc.dram_tensor("sink", [128, NT * S], BF16, kind="ExternalInput")
    pmat_d = nc.dram_tensor("pmat", [128, 128], BF16, kind="ExternalInput")
    ones8_d = nc.dram_tensor("ones8", [128, 256], FP8, kind="ExternalInput")
    out_d = nc.dram_tensor("out", [S, E], F32, kind="ExternalOutput")

    SCALE = 1.0 / float(np.sqrt(HD))

    def r3(ap):
        return ap.rearrange("p (t s) -> p t s", t=NT)

    with tile.TileContext(nc) as tc:
      from contextlib import ExitStack

      with ExitStack() as top:
        res = top.enter_context(tc.tile_pool(name="res", bufs=1))
        consts = top.enter_context(tc.tile_pool(name="consts", bufs=1))
        wp8 = top.enter_context(tc.tile_pool(name="wp8", bufs=4))
        wpb = top.enter_context(tc.tile_pool(name="wpb", bufs=2))
        psum = top.enter_context(tc.tile_pool(name="psum", bufs=1, space="PSUM"))

        def load_w8(nm):
            t = wp8.tile([128, NT, E], FP8, tag="W8", name=f"w_{nm}")
            nc.sync.dma_start(out=t, in_=w8_d[nm].ap().rearrange("p (t e) -> p t e", t=NT))
            return t

        def load_wb(nm, tag="Wb"):
            t = wpb.tile([128, NT, E], BF16, tag=tag, name=f"w_{nm}", bufs=(1 if tag == "Wb2" else None))
            nc.sync.dma_start(out=t, in_=wb_d[nm].ap().rearrange("p (t e) -> p t e", t=NT))
            return t

        # ------- loads in prefetch order (tiny consts first: the P2 bias
        # matmul sits early in PE program order, so bv_row must land first
        # or it head-of-line-blocks the whole PE queue) -------------------
        pmat = consts.tile([128, 128], BF16, tag="pmat")
        nc.sync.dma_start(out=pmat, in_=pmat_d.ap())
        bq_sb = consts.tile([128, NT], F32, tag="bq")
        nc.sync.dma_start(out=bq_sb, in_=bq_d.ap())
        bk_sb = consts.tile([128, NT], F32, tag="bk")
        nc.sync.dma_start(out=bk_sb, in_=bk_d.ap())
        bo8_sb = consts.tile([128, NT], F32, tag="bo8")
        nc.sync.dma_start(out=bo8_sb, in_=bo8_d.ap())
        bv_row = consts.tile([1, E], BF16, tag="bv_row")
        nc.sync.dma_start(out=bv_row, in_=bv_row_d.ap())
        bxz_row = consts.tile([1, E], BF16, tag="bxz_row")
        nc.sync.dma_start(out=bxz_row, in_=bxz_row_d.ap())
        ones8 = consts.tile([128, 2, 128], FP8, tag="ones8")  # value 1/CS
        nc.sync.dma_start(out=ones8, in_=ones8_d.ap().rearrange("p (a b) -> p a b", a=2))
        ones1_b = consts.tile([1, 128], BF16, tag="ones1_b")
        nc.vector.memset(ones1_b, 1.0)

        xT8 = res.tile([128, NT, S], FP8, tag="xT8")
        xt8_r = r3(xt8_d.ap())
        nc.sync.dma_start(out=xT8[:, 0:2, :], in_=xt8_r[:, 0:2, :])
        wv_sb = wp8.tile([128, NT, E], FP8, tag="W8", name="w_Wv")
        wv_r = w8_d["Wv"].ap().rearrange("p (t e) -> p t e", t=NT)
        nc.sync.dma_start(out=wv_sb[:, :, 0:512], in_=wv_r[:, :, 0:512])
        nc.sync.dma_start(out=xT8[:, 2:NT, :], in_=xt8_r[:, 2:NT, :])
        nc.sync.dma_start(out=wv_sb[:, :, 512:E], in_=wv_r[:, :, 512:E])
        wq_sb = load_w8("Wq")
        wk_sb = load_w8("Wk")

        # mid tiles live through P4 only
        mid_ctx = tc.tile_pool(name="mid", bufs=1)
        mid = mid_ctx.__enter__()
        vsb8 = mid.tile([128, NT, E], FP8, tag="vsb8")   # v in fp8  [s, e]
        qr = mid.tile([128, NT, S], BF16, tag="qr")      # rope(q)^T
        kr = mid.tile([128, NT, S], BF16, tag="kr")      # rope(k)^T
        # rope tables live through P3 only
        tab_ctx = tc.tile_pool(name="tab", bufs=1)
        tab = tab_ctx.__enter__()
        cosq = tab.tile([128, NT, S], BF16, tag="cosq")
        nc.sync.dma_start(out=cosq, in_=r3(cosq_d.ap()))
        sinq = tab.tile([128, NT, S], BF16, tag="sinq")
        nc.sync.dma_start(out=sinq, in_=r3(sinq_d.ap()))

        xTb = res.tile([128, NT, S], BF16, tag="xTb")
        nc.sync.dma_start(out=xTb, in_=r3(xtb_d.ap()))
        # prefetch the rest of the weights (slots free up as phases finish)
        wo_sb = load_w8("Wo")
        wxr_sb = load_w8("Wxr")
        wyr_sb = load_w8("Wyr")
        wxz_sb = load_wb("Wxz")
        wyz_sb = load_w8("Wyz")
        wxg_sb = load_wb("Wxg", tag="Wb2")
        wyg_sb = load_w8("Wyg")

        # =========== P2: v = (x @ Wv) + bv  (seq-major, fp8 out) ============
        for st in range(NT):
            ss = slice(st * 128, (st + 1) * 128)
            for c in range(2):
                sl = slice(c * 512, (c + 1) * 512)
                ps = psum.tile([128, 512], F32, tag="mm", bufs=3, name="ps_v")
                for kp in range(NP):
                    nc.tensor.matmul(
                        ps, lhsT=xT8[:, 2 * kp:2 * kp + 2, ss],
                        rhs=wv_sb[:, 2 * kp:2 * kp + 2, sl],
                        start=(kp == 0), stop=False, perf_mode=DR)
                nc.tensor.matmul(ps, lhsT=ones1_b, rhs=bv_row[:, sl],
                                 start=False, stop=True)
                nc.vector.tensor_scalar(out=vsb8[:, st, sl], in0=ps,
                                        scalar1=1.0 / WS, scalar2=None, op0=ALU.mult)

        # =========== P3: q/k proj (fp8 DR) + RoPE (bf16) =====================
        with tc.tile_pool(name="p3", bufs=4) as p3:
            def proj_tile(t, w_sb, bias_sb):
                qs = p3.tile([128, S], BF16, tag="qs")
                for c in range(2):
                    sl = slice(c * 512, (c + 1) * 512)
                    ps = psum.tile([128, 512], F32, tag="mm", bufs=3, name="ps_qk")
                    for kp in range(NP):
                        nc.tensor.matmul(
                            ps, lhsT=w_sb[:, 2 * kp:2 * kp + 2, t * 128:(t + 1) * 128],
                            rhs=xT8[:, 2 * kp:2 * kp + 2, sl],
                            start=(kp == 0), stop=(kp == NP - 1), perf_mode=DR)
                    nc.scalar.activation(qs[:, sl], ps, AF.Identity,
                                         bias=bias_sb[:, t:t + 1], scale=1.0 / WS)
                return qs

            def rotate(t, qs, cos_t, sin_t, dst):
                for c in range(2):
                    sl = slice(c * 512, (c + 1) * 512)
                    sh = psum.tile([128, 512], F32, tag="mm", bufs=3, name="ps_sh")
                    nc.tensor.matmul(sh, lhsT=pmat, rhs=qs[:, sl], start=True, stop=True)
                    shb = p3.tile([128, 512], BF16, tag="shb", bufs=2)
                    nc.scalar.copy(shb, sh)
                    t1 = p3.tile([128, 512], BF16, tag="t1", bufs=3)
                    nc.vector.tensor_mul(t1, qs[:, sl], cos_t[:, t, sl])
                    t2 = p3.tile([128, 512], BF16, tag="t2", bufs=3)
                    nc.vector.tensor_mul(t2, shb, sin_t[:, t, sl])
                    nc.gpsimd.tensor_add(dst[:, t, sl], t1, t2)

            if share_qk:
                prev = None
                for t in range(NT):
                    qs = proj_tile(t, wq_sb, bq_sb)
                    ks = proj_tile(t, wk_sb, bk_sb)
                    if prev is not None:
                        pt, pq, pk = prev
                        rotate(pt, pq, cosq, sinq, qr)
                        rotate(pt, pk, cosq, sinq, kr)
                    prev = (t, qs, ks)
                pt, pq, pk = prev
                rotate(pt, pq, cosq, sinq, qr)
                rotate(pt, pk, cosq, sinq, kr)
            else:
                # correctness-first fallback: q pass, reload tables, k pass
                for t in range(NT):
                    qs = proj_tile(t, wq_sb, bq_sb)
                    rotate(t, qs, cosq, sinq, qr)
                cosk = tab.tile([128, NT, S], BF16, tag="cosq")
                nc.sync.dma_start(out=cosk, in_=r3(cosk_d.ap()))
                sink = tab.tile([128, NT, S], BF16, tag="sinq")
                nc.sync.dma_start(out=sink, in_=r3(sink_d.ap()))
                for t in range(NT):
                    ks = proj_tile(t, wk_sb, bk_sb)
                    rotate(t, ks, cosk, sink, kr)

        # =========== P4: attention per head ==================================
        tab_ctx.__exit__(None, None, None)
        with tc.tile_pool(name="p4", bufs=2) as p4:
            ctx8 = res.tile([128, NT, S], FP8, tag="ctx8")  # 16*ctx in fp8

            def emit_scores(h):
                expT = p4.tile([128, NT, S], FP8, tag="expT", bufs=3)
                for jt in range(NT):
                    i0 = jt * 128
                    ps = psum.tile([128, 1024], F32, tag="sc", bufs=2, name="ps_sc")
                    if i0 < 512:
                        nc.tensor.matmul(
                            ps[:, i0:512],
                            lhsT=kr[:, h, i0:i0 + 128],
                            rhs=qr[:, h, i0:512], start=True, stop=True)
                        nc.tensor.matmul(
                            ps[:, 512:1024],
                            lhsT=kr[:, h, i0:i0 + 128],
                            rhs=qr[:, h, 512:1024], start=True, stop=True)
                    else:
                        nc.tensor.matmul(
                            ps[:, i0:1024],
                            lhsT=kr[:, h, i0:i0 + 128],
                            rhs=qr[:, h, i0:1024], start=True, stop=True)
                    nc.scalar.activation(expT[:, jt, i0:1024], ps[:, i0:1024],
                                         AF.Exp, scale=SCALE)
                    nc.gpsimd.affine_select(
                        out=expT[:, jt, i0:i0 + 128], in_=expT[:, jt, i0:i0 + 128],
                        pattern=[[1, 128]], compare_op=ALU.is_ge,
                        fill=0.0, base=0, channel_multiplier=-1)
                    if jt % 2 == 1:
                        nc.vector.memset(expT[:, jt, i0 - 128:i0], 0.0)
                return expT

            def emit_av(h, expT):
                for c in range(2):
                    cs, ce = c * 512, (c + 1) * 512
                    jps = [jp for jp in range(NP) if jp * 256 < ce]
                    dps = psum.tile([128, 512], F32, tag="dc", bufs=1, name="ps_den")
                    for n, jp in enumerate(jps):
                        a = max(jp * 256, cs)
                        nc.tensor.matmul(
                            dps[:, a - cs:512], lhsT=ones8,
                            rhs=expT[:, 2 * jp:2 * jp + 2, a:ce],
                            start=(n == 0), stop=(n == len(jps) - 1), perf_mode=DR)
                    rf = p4.tile([128, 512], F32, tag="rf")
                    nc.vector.reciprocal_approx_fast(out=rf, in_=dps)
                    cps = psum.tile([128, 512], F32, tag="dc", bufs=1, name="ps_ctx")
                    for n, jp in enumerate(jps):
                        a = max(jp * 256, cs)
                        nc.tensor.matmul(
                            cps[:, a - cs:512],
                            lhsT=vsb8[:, 2 * jp:2 * jp + 2, h * 128:(h + 1) * 128],
                            rhs=expT[:, 2 * jp:2 * jp + 2, a:ce],
                            start=(n == 0), stop=(n == len(jps) - 1), perf_mode=DR)
                    nc.vector.tensor_mul(ctx8[:, h, cs:ce], cps, rf)

            prev_h = None
            for h in range(H):
                expT = emit_scores(h)
                if prev_h is not None:
                    emit_av(prev_h[0], prev_h[1])
                prev_h = (h, expT)
            emit_av(prev_h[0], prev_h[1])

        # =========== P5: y8 = 8*relu(ctx Wo + bo)  (feature-major) ==========
        mid_ctx.__exit__(None, None, None)
        res2 = top.enter_context(tc.tile_pool(name="res2", bufs=1))
        yT8 = res2.tile([128, NT, S], FP8, tag="yT8")    # 8*y in fp8
        rx = res2.tile([128, NT, S], BF16, tag="rx")     # (r*x)^T bf16
        for t in range(NT):
            for c in range(2):
                sl = slice(c * 512, (c + 1) * 512)
                ps = psum.tile([128, 512], F32, tag="mm", bufs=3, name="ps_y")
                for kp in range(NP):
                    nc.tensor.matmul(
                        ps, lhsT=wo_sb[:, 2 * kp:2 * kp + 2, t * 128:(t + 1) * 128],
                        rhs=ctx8[:, 2 * kp:2 * kp + 2, sl],
                        start=(kp == 0), stop=(kp == NP - 1), perf_mode=DR)
                # psum = WS*CS*(ctx@Wo); y8 = relu(psum*YS/(WS*CS) + YS*bo)
                nc.scalar.activation(yT8[:, t, sl], ps, AF.Relu,
                                     bias=bo8_sb[:, t:t + 1],
                                     scale=YS / (WS * CS))

        # ===== P6: r = sigmoid(x Wxr + y Wyr); rx = r * xT (bf16) ============
        with tc.tile_pool(name="p6", bufs=3) as p6:
            for t in range(NT):
                for c in range(2):
                    sl = slice(c * 512, (c + 1) * 512)
                    ps = psum.tile([128, 512], F32, tag="mm", bufs=3, name="ps_r")
                    for kp in range(NP):
                        nc.tensor.matmul(
                            ps, lhsT=wxr_sb[:, 2 * kp:2 * kp + 2, t * 128:(t + 1) * 128],
                            rhs=xT8[:, 2 * kp:2 * kp + 2, sl],
                            start=(kp == 0), stop=False, perf_mode=DR)
                    for kp in range(NP):
                        nc.tensor.matmul(
                            ps, lhsT=wyr_sb[:, 2 * kp:2 * kp + 2, t * 128:(t + 1) * 128],
                            rhs=yT8[:, 2 * kp:2 * kp + 2, sl],
                            start=False, stop=(kp == NP - 1), perf_mode=DR)
                    rt = p6.tile([128, 512], BF16, tag="rt")
                    nc.scalar.activation(rt, ps, AF.Sigmoid, scale=1.0 / WS)
                    nc.vector.tensor_mul(rx[:, t, sl], rt, xTb[:, t, sl])

        # =========== P7: z/h + gated combine (seq-major, single pass) ========
        with tc.tile_pool(name="p7", bufs=2) as p7:
            for st in range(NT):
                ss = slice(st * 128, (st + 1) * 128)
                xf = p7.tile([128, E], F32, tag="xf")
                nc.sync.dma_start(out=xf, in_=xb_d.ap()[ss, :])
                ot = p7.tile([128, E], F32, tag="ot")
                for c in range(2):
                    sl = slice(c * 512, (c + 1) * 512)
                    zps = psum.tile([128, 512], F32, tag="mm", bufs=3, name="ps_z")
                    for kt in range(NT):
                        nc.tensor.matmul(zps, lhsT=xTb[:, kt, ss],
                                         rhs=wxz_sb[:, kt, sl],
                                         start=(kt == 0), stop=False)
                    for kp in range(NP):
                        nc.tensor.matmul(zps, lhsT=yT8[:, 2 * kp:2 * kp + 2, ss],
                                         rhs=wyz_sb[:, 2 * kp:2 * kp + 2, sl],
                                         start=False, stop=False, perf_mode=DR)
                    nc.tensor.matmul(zps, lhsT=ones1_b, rhs=bxz_row[:, sl],
                                     start=False, stop=True)
                    zt = p7.tile([128, 512], F32, tag="zt")
                    nc.scalar.activation(zt, zps, AF.Sigmoid, scale=1.0 / WS)
                    hps = psum.tile([128, 512], F32, tag="mm", bufs=3, name="ps_h")
                    for kt in range(NT):
                        nc.tensor.matmul(hps, lhsT=rx[:, kt, ss],
                                         rhs=wxg_sb[:, kt, sl],
                                         start=(kt == 0), stop=False)
                    for kp in range(NP):
                        nc.tensor.matmul(hps, lhsT=yT8[:, 2 * kp:2 * kp + 2, ss],
                                         rhs=wyg_sb[:, 2 * kp:2 * kp + 2, sl],
                                         start=False, stop=(kp == NP - 1), perf_mode=DR)
                    ht = p7.tile([128, 512], F32, tag="ht")
                    nc.scalar.activation(ht, hps, AF.Tanh, scale=1.0 / WS)
                    dt = p7.tile([128, 512], F32, tag="dt")
                    nc.vector.tensor_sub(dt, ht, xf[:, sl])
                    zd = p7.tile([128, 512], F32, tag="zd")
                    nc.vector.tensor_mul(zd, zt, dt)
                    nc.vector.tensor_add(ot[:, sl], xf[:, sl], zd)
                nc.sync.dma_start(out=out_d.ap()[ss, :], in_=ot)

    nc.compile()
    return nc


# ---------------- host-side packing -----------------------------------------

def _pack_w(w, scale, npdt):
    return np.ascontiguousarray(
        (np.asarray(w, np.float32) * scale).astype(npdt)
        .reshape(NT, 128, E).transpose(1, 0, 2).reshape(128, NT * E))


def _pack_fm(m, npdt):
    # [E, S]-logical feature-major -> [128, NT*S]
    return np.ascontiguousarray(
        m.astype(npdt).reshape(NT, 128, S).transpose(1, 0, 2).reshape(128, NT * S))


def _pack_bias_fm(b, scale=1.0):
    return np.ascontiguousarray(
        (np.asarray(b, np.float32) * scale).reshape(NT, 128).T)


_INV = None


def _inv_pair():
    global _INV
    if _INV is None:
        inv = 1.0 / (10000.0 ** (np.arange(0, E, 2, dtype=np.float32) / np.float32(E)))
        _INV = np.repeat(inv.astype(np.float64), 2)  # pair-expanded [E]
    return _INV


def _tables(idx):
    f = _inv_pair()[:, None] * idx.astype(np.float64)[None, :]  # [E, S]
    return (_pack_fm(np.cos(f).astype(np.float32), NPBF16),
            _pack_fm(np.sin(f).astype(np.float32), NPBF16))


def _pmat():
    pm = np.zeros((128, 128), dtype=NPBF16)
    for i in range(64):
        pm[2 * i + 1, 2 * i] = -1.0
        pm[2 * i, 2 * i + 1] = 1.0
    return pm


def make_in_maps(inputs, share_qk):
    x = np.asarray(inputs["x"], dtype=np.float32)
    qi = np.asarray(inputs["query_index"])
    ki = np.asarray(inputs["key_index"])
    common = {
        "bq": _pack_bias_fm(np.asarray(inputs["bq"])),
        "bk": _pack_bias_fm(np.asarray(inputs["bk"])),
        "bo8": _pack_bias_fm(np.asarray(inputs["bo"]), YS),
        "bv_row": (np.asarray(inputs["bv"], np.float32) * WS).astype(NPBF16).reshape(1, E),
        "bxz_row": (np.asarray(inputs["bxz"], np.float32) * WS).astype(NPBF16).reshape(1, E),
        "ones8": np.full((128, 256), 1.0 / CS, NPFP8),
        "pmat": _pmat(),
    }
    for nm in ("Wq", "Wk", "Wv", "Wo", "Wxr"):
        common[nm] = _pack_w(inputs[nm], WS, NPFP8)
    for nm in ("Wyr", "Wyz", "Wyg"):
        common[nm] = _pack_w(inputs[nm], WS / YS, NPFP8)
    for nm in ("Wxz", "Wxg"):
        common[nm] = _pack_w(inputs[nm], WS, NPBF16)
    in_maps = []
    for b in range(B):
        m = dict(common)
        xb = np.ascontiguousarray(x[b])
        m["xb"] = xb
        xt = xb.T  # [E, S]
        m["xtb"] = _pack_fm(xt, NPBF16)
        m["xt8"] = _pack_fm(xt, NPFP8)
        m["cosq"], m["sinq"] = _tables(qi[b])
        if not share_qk:
            m["cosk"], m["sink"] = _tables(ki[b])
        in_maps.append(m)
    return in_maps


def kernel(**inputs):
    qi = np.asarray(inputs["query_index"])
    ki = np.asarray(inputs["key_index"])
    share_qk = bool(np.array_equal(qi, ki))

    key = ("k", share_qk)
    if key not in _COMPILED:
        _COMPILED[key] = _build(share_qk)
    nc = _COMPILED[key]

    in_maps = make_in_maps(inputs, share_qk)
    global _dbg_in_maps
    _dbg_in_maps = in_maps
    res = bass_utils.run_bass_kernel_spmd(nc, in_maps, core_ids=list(range(NC)))
    out = np.stack([res.results[b]["out"] for b in range(B)]).astype(np.float32)
    return out


# revision 46
# speedup vs baseline: 1.3043x; 1.1232x over previous
# Trainium2 Bass kernel for nn_EpisodeMultiheadAttentionBlock.
# B=8, S=1024, E=1024, H=8 heads, HD=128. Data-parallel over batch: core b
# computes batch element b. Self-contained: only needs /opt/trn_rl_repo on path.
#
# v3: fp8(e4m3) DoubleRow matmuls for QKV/Wo/y-side gates and attention AV/den
# (weights pre-scaled by powers of 2, dequantized via activation scales);
# host-precomputed x^T and RoPE cos/sin tables; bf16 where precision demands
# it (scores, x-side gate matmuls, rope rotation). Single long-lived PSUM
# pool (no phase barriers), DMA loads ordered for prefetch, elementwise work
# spread across DVE/Act/Pool.
import sys
import numpy as np

sys.path.insert(0, "/opt/trn_rl_repo")

import ml_dtypes  # noqa: E402
import concourse.bass as bass  # noqa: E402
import concourse.mybir as mybir  # noqa: E402
import concourse.tile as tile  # noqa: E402
from concourse import bacc  # noqa: E402
from concourse import bass_utils  # noqa: E402

B, S, E, H = 8, 1024, 1024, 8
HD = E // H  # 128
NT = E // 128  # 8 e-tiles / s-tiles
NP = NT // 2  # 4 DoubleRow k-tile pairs
NC = 8  # cores
BF16 = mybir.dt.bfloat16
F32 = mybir.dt.float32
FP8 = mybir.dt.float8e4
AF = mybir.ActivationFunctionType
DR = mybir.MatmulPerfMode.DoubleRow
ALU = mybir.AluOpType
NPBF16 = ml_dtypes.bfloat16
NPFP8 = ml_dtypes.float8_e4m3

WS = 32.0  # weight pre-scale for fp8/bf16 weights
YS = 8.0   # y stored as 8*y in fp8
CS = 16.0  # ctx stored as 16*ctx in fp8

_COMPILED = {}


def _build(share_qk: bool):
    nc = bacc.Bacc("TRN2", target_bir_lowering=False, debug=False, num_devices=NC)

    # ---- DRAM tensors -------------------------------------------------------
    xb_d = nc.dram_tensor("xb", [S, E], F32, kind="ExternalInput")
    xtb_d = nc.dram_tensor("xtb", [128, NT * S], BF16, kind="ExternalInput")
    xt8_d = nc.dram_tensor("xt8", [128, NT * S], FP8, kind="ExternalInput")
    xl8_d = nc.dram_tensor("xl8", [128, NT * S], FP8, kind="ExternalInput")
    w8_d = {
        nm: nc.dram_tensor(nm, [128, NT * E], FP8, kind="ExternalInput")
        for nm in ("Wq", "Wk", "Wv", "Wo", "Wxr", "Wxg", "Wyr", "Wyz", "Wyg",
                   "Wxzh", "Wxzh2", "WxzR")
    }
    bq_d = nc.dram_tensor("bq", [128, NT], F32, kind="ExternalInput")
    bk_d = nc.dram_tensor("bk", [128, NT], F32, kind="ExternalInput")
    bo8_d = nc.dram_tensor("bo8", [128, NT], F32, kind="ExternalInput")
    bv_row_d = nc.dram_tensor("bv_row", [1, E], BF16, kind="ExternalInput")   # 32*bv
    bxz_row_d = nc.dram_tensor("bxz_row", [1, E], BF16, kind="ExternalInput")  # 32*bxz
    cosq_d = nc.dram_tensor("cosq", [128, NT * S], BF16, kind="ExternalInput")
    sinq_d = nc.dram_tensor("sinq", [128, NT * S], BF16, kind="ExternalInput")
    if not share_qk:
        cosk_d = nc.dram_tensor("cosk", [128, NT * S], BF16, kind="ExternalInput")
        sink_d = nc.dram_tensor("sink", [128, NT * S], BF16, kind="ExternalInput")
    pmat_d = nc.dram_tensor("pmat", [128, 128], BF16, kind="ExternalInput")
    ones8_d = nc.dram_tensor("ones8", [128, 256], FP8, kind="ExternalInput")
    out_d = nc.dram_tensor("out", [S, E], F32, kind="ExternalOutput")

    SCALE = 1.0 / float(np.sqrt(HD))

    def r3(ap):
        return ap.rearrange("p (t s) -> p t s", t=NT)

    with tile.TileContext(nc) as tc:
      from contextlib import ExitStack

      with ExitStack() as top:
        res = top.enter_context(tc.tile_pool(name="res", bufs=1))
        consts = top.enter_context(tc.tile_pool(name="consts", bufs=1))
        wp8 = top.enter_context(tc.tile_pool(name="wp8", bufs=6))
        psum = top.enter_context(tc.tile_pool(name="psum", bufs=1, space="PSUM"))

        def load_w8(nm):
            t = wp8.tile([128, NT, E], FP8, tag="W8", name=f"w_{nm}")
            nc.sync.dma_start(out=t, in_=w8_d[nm].ap().rearrange("p (t e) -> p t e", t=NT))
            return t

        # ------- loads in prefetch order (tiny consts first: the P2 bias
        # matmul sits early in PE program order, so bv_row must land first
        # or it head-of-line-blocks the whole PE queue) -------------------
        pmat = consts.tile([128, 128], BF16, tag="pmat")
        nc.sync.dma_start(out=pmat, in_=pmat_d.ap())
        bq_sb = consts.tile([128, NT], F32, tag="bq")
        nc.sync.dma_start(out=bq_sb, in_=bq_d.ap())
        bk_sb = consts.tile([128, NT], F32, tag="bk")
        nc.sync.dma_start(out=bk_sb, in_=bk_d.ap())
        bo8_sb = consts.tile([128, NT], F32, tag="bo8")
        nc.sync.dma_start(out=bo8_sb, in_=bo8_d.ap())
        bv_row = consts.tile([1, E], BF16, tag="bv_row")
        nc.sync.dma_start(out=bv_row, in_=bv_row_d.ap())
        bxz_row = consts.tile([1, E], BF16, tag="bxz_row")
        nc.sync.dma_start(out=bxz_row, in_=bxz_row_d.ap())
        ones8 = consts.tile([128, 2, 128], FP8, tag="ones8")  # value 1/CS
        nc.sync.dma_start(out=ones8, in_=ones8_d.ap().rearrange("p (a b) -> p a b", a=2))
        ones1_b = consts.tile([1, 128], BF16, tag="ones1_b")
        nc.vector.memset(ones1_b, 1.0)

        xT8 = res.tile([128, NT, S], FP8, tag="xT8")
        xt8_r = r3(xt8_d.ap())
        nc.sync.dma_start(out=xT8[:, 0:2, :], in_=xt8_r[:, 0:2, :])
        wv_sb = wp8.tile([128, NT, E], FP8, tag="W8", name="w_Wv")
        wv_r = w8_d["Wv"].ap().rearrange("p (t e) -> p t e", t=NT)
        nc.sync.dma_start(out=wv_sb[:, :, 0:512], in_=wv_r[:, :, 0:512])
        nc.sync.dma_start(out=xT8[:, 2:NT, :], in_=xt8_r[:, 2:NT, :])
        nc.sync.dma_start(out=wv_sb[:, :, 512:E], in_=wv_r[:, :, 512:E])
        wq_sb = load_w8("Wq")
        wk_sb = load_w8("Wk")

        # mid tiles live through P4 only
        mid_ctx = tc.tile_pool(name="mid", bufs=1)
        mid = mid_ctx.__enter__()
        vsb8 = mid.tile([128, NT, E], FP8, tag="vsb8")   # v in fp8  [s, e]
        qr = mid.tile([128, NT, S], BF16, tag="qr")      # rope(q)^T
        kr = mid.tile([128, NT, S], BF16, tag="kr")      # rope(k)^T
        # rope tables live through P3 only
        tab_ctx = tc.tile_pool(name="tab", bufs=1)
        tab = tab_ctx.__enter__()
        cosq = tab.tile([128, NT, S], BF16, tag="cosq")
        nc.sync.dma_start(out=cosq, in_=r3(cosq_d.ap()))
        sinq = tab.tile([128, NT, S], BF16, tag="sinq")
        nc.sync.dma_start(out=sinq, in_=r3(sinq_d.ap()))

        xTb = res.tile([128, NT, S], BF16, tag="xTb")
        nc.sync.dma_start(out=xTb, in_=r3(xtb_d.ap()))
        xl8 = res.tile([128, NT, S], FP8, tag="xl8")
        nc.sync.dma_start(out=xl8, in_=r3(xl8_d.ap()))
        # prefetch the rest of the weights (slots free up as phases finish)
        wo_sb = load_w8("Wo")
        wxr_sb = load_w8("Wxr")
        wyr_sb = load_w8("Wyr")
        wxzh_sb = load_w8("Wxzh")
        wxzh2_sb = load_w8("Wxzh2")
        wxzr_sb = load_w8("WxzR")
        wyz_sb = load_w8("Wyz")
        wxg_sb = load_w8("Wxg")
        wyg_sb = load_w8("Wyg")

        # =========== P2: v = (x @ Wv) + bv  (seq-major, fp8 out) ============
        for st in range(NT):
            ss = slice(st * 128, (st + 1) * 128)
            for c in range(2):
                sl = slice(c * 512, (c + 1) * 512)
                ps = psum.tile([128, 512], F32, tag="mm", bufs=3, name="ps_v")
                for kp in range(NP):
                    nc.tensor.matmul(
                        ps, lhsT=xT8[:, 2 * kp:2 * kp + 2, ss],
                        rhs=wv_sb[:, 2 * kp:2 * kp + 2, sl],
                        start=(kp == 0), stop=False, perf_mode=DR)
                nc.tensor.matmul(ps, lhsT=ones1_b, rhs=bv_row[:, sl],
                                 start=False, stop=True)
                nc.vector.tensor_scalar(out=vsb8[:, st, sl], in0=ps,
                                        scalar1=1.0 / WS, scalar2=None, op0=ALU.mult)

        # =========== P3: q/k proj (fp8 DR) + RoPE (bf16) =====================
        with tc.tile_pool(name="p3", bufs=4) as p3:
            def proj_tile(t, w_sb, bias_sb):
                qs = p3.tile([128, S], BF16, tag="qs")
                for c in range(2):
                    sl = slice(c * 512, (c + 1) * 512)
                    ps = psum.tile([128, 512], F32, tag="mm", bufs=3, name="ps_qk")
                    for kp in range(NP):
                        nc.tensor.matmul(
                            ps, lhsT=w_sb[:, 2 * kp:2 * kp + 2, t * 128:(t + 1) * 128],
                            rhs=xT8[:, 2 * kp:2 * kp + 2, sl],
                            start=(kp == 0), stop=(kp == NP - 1), perf_mode=DR)
                    nc.vector.tensor_scalar(out=qs[:, sl], in0=ps,
                                            scalar1=1.0 / WS, scalar2=bias_sb[:, t:t + 1],
                                            op0=ALU.mult, op1=ALU.add)
                return qs

            def rotate(t, qs, cos_t, sin_t, dst):
                for c in range(2):
                    sl = slice(c * 512, (c + 1) * 512)
                    sh = psum.tile([128, 512], F32, tag="mm", bufs=3, name="ps_sh")
                    nc.tensor.matmul(sh, lhsT=pmat, rhs=qs[:, sl], start=True, stop=True)
                    shb = p3.tile([128, 512], BF16, tag="shb", bufs=2)
                    nc.scalar.copy(shb, sh)
                    t1 = p3.tile([128, 512], BF16, tag="t1", bufs=3)
                    nc.vector.tensor_mul(t1, qs[:, sl], cos_t[:, t, sl])
                    t2 = p3.tile([128, 512], BF16, tag="t2", bufs=3)
                    nc.vector.tensor_mul(t2, shb, sin_t[:, t, sl])
                    nc.gpsimd.tensor_add(dst[:, t, sl], t1, t2)

            if share_qk:
                prev = None
                for t in range(NT):
                    qs = proj_tile(t, wq_sb, bq_sb)
                    ks = proj_tile(t, wk_sb, bk_sb)
                    if prev is not None:
                        pt, pq, pk = prev
                        rotate(pt, pq, cosq, sinq, qr)
                        rotate(pt, pk, cosq, sinq, kr)
                    prev = (t, qs, ks)
                pt, pq, pk = prev
                rotate(pt, pq, cosq, sinq, qr)
                rotate(pt, pk, cosq, sinq, kr)
            else:
                # correctness-first fallback: q pass, reload tables, k pass
                for t in range(NT):
                    qs = proj_tile(t, wq_sb, bq_sb)
                    rotate(t, qs, cosq, sinq, qr)
                cosk = tab.tile([128, NT, S], BF16, tag="cosq")
                nc.sync.dma_start(out=cosk, in_=r3(cosk_d.ap()))
                sink = tab.tile([128, NT, S], BF16, tag="sinq")
                nc.sync.dma_start(out=sink, in_=r3(sink_d.ap()))
                for t in range(NT):
                    ks = proj_tile(t, wk_sb, bk_sb)
                    rotate(t, ks, cosk, sink, kr)

        # =========== P4: attention per head ==================================
        tab_ctx.__exit__(None, None, None)
        with tc.tile_pool(name="p4", bufs=2) as p4:
            ctx8 = res.tile([128, NT, S], FP8, tag="ctx8")  # 16*ctx in fp8

            def emit_scores(h):
                expT = p4.tile([128, NT, S], FP8, tag="expT", bufs=3)
                for jt in range(NT):
                    i0 = jt * 128
                    ps = psum.tile([128, 1024], F32, tag="sc", bufs=2, name="ps_sc")
                    if i0 < 512:
                        nc.tensor.matmul(
                            ps[:, i0:512],
                            lhsT=kr[:, h, i0:i0 + 128],
                            rhs=qr[:, h, i0:512], start=True, stop=True)
                        nc.tensor.matmul(
                            ps[:, 512:1024],
                            lhsT=kr[:, h, i0:i0 + 128],
                            rhs=qr[:, h, 512:1024], start=True, stop=True)
                    else:
                        nc.tensor.matmul(
                            ps[:, i0:1024],
                            lhsT=kr[:, h, i0:i0 + 128],
                            rhs=qr[:, h, i0:1024], start=True, stop=True)
                    nc.scalar.activation(expT[:, jt, i0:1024], ps[:, i0:1024],
                                         AF.Exp, scale=SCALE)
                    nc.gpsimd.affine_select(
                        out=expT[:, jt, i0:i0 + 128], in_=expT[:, jt, i0:i0 + 128],
                        pattern=[[1, 128]], compare_op=ALU.is_ge,
                        fill=0.0, base=0, channel_multiplier=-1)
                    if jt % 2 == 1:
                        nc.vector.memset(expT[:, jt, i0 - 128:i0], 0.0)
                return expT

            def emit_av(h, expT):
                for c in range(2):
                    cs, ce = c * 512, (c + 1) * 512
                    jps = [jp for jp in range(NP) if jp * 256 < ce]
                    dps = psum.tile([128, 512], F32, tag="dc", bufs=1, name="ps_den")
                    for n, jp in enumerate(jps):
                        a = max(jp * 256, cs)
                        nc.tensor.matmul(
                            dps[:, a - cs:512], lhsT=ones8,
                            rhs=expT[:, 2 * jp:2 * jp + 2, a:ce],
                            start=(n == 0), stop=(n == len(jps) - 1), perf_mode=DR)
                    rf = p4.tile([128, 512], F32, tag="rf")
                    nc.vector.reciprocal_approx_fast(out=rf, in_=dps)
                    cps = psum.tile([128, 512], F32, tag="dc", bufs=1, name="ps_ctx")
                    for n, jp in enumerate(jps):
                        a = max(jp * 256, cs)
                        nc.tensor.matmul(
                            cps[:, a - cs:512],
                            lhsT=vsb8[:, 2 * jp:2 * jp + 2, h * 128:(h + 1) * 128],
                            rhs=expT[:, 2 * jp:2 * jp + 2, a:ce],
                            start=(n == 0), stop=(n == len(jps) - 1), perf_mode=DR)
                    nc.vector.tensor_mul(ctx8[:, h, cs:ce], cps, rf)

            prev_h = None
            for h in range(H):
                expT = emit_scores(h)
                if prev_h is not None:
                    emit_av(prev_h[0], prev_h[1])
                prev_h = (h, expT)
            emit_av(prev_h[0], prev_h[1])

        # =========== P5: y8 = 8*relu(ctx Wo + bo)  (feature-major) ==========
        mid_ctx.__exit__(None, None, None)
        res2 = top.enter_context(tc.tile_pool(name="res2", bufs=1))
        yT8 = res2.tile([128, NT, S], FP8, tag="yT8")    # 8*y in fp8
        rx = res2.tile([128, NT, S], FP8, tag="rx")      # (r*x)^T fp8
        for t in range(NT):
            for c in range(2):
                sl = slice(c * 512, (c + 1) * 512)
                ps = psum.tile([128, 512], F32, tag="mm", bufs=3, name="ps_y")
                for kp in range(NP):
                    nc.tensor.matmul(
                        ps, lhsT=wo_sb[:, 2 * kp:2 * kp + 2, t * 128:(t + 1) * 128],
                        rhs=ctx8[:, 2 * kp:2 * kp + 2, sl],
                        start=(kp == 0), stop=(kp == NP - 1), perf_mode=DR)
                # psum = WS*CS*(ctx@Wo); y8 = relu(psum*YS/(WS*CS) + YS*bo)
                nc.scalar.activation(yT8[:, t, sl], ps, AF.Relu,
                                     bias=bo8_sb[:, t:t + 1],
                                     scale=YS / (WS * CS))

        # ===== P6: r = sigmoid(x Wxr + y Wyr); rx = r * xT (bf16) ============
        with tc.tile_pool(name="p6", bufs=3) as p6:
            for t in range(NT):
                for c in range(2):
                    sl = slice(c * 512, (c + 1) * 512)
                    ps = psum.tile([128, 512], F32, tag="mm", bufs=3, name="ps_r")
                    for kp in range(NP):
                        nc.tensor.matmul(
                            ps, lhsT=wxr_sb[:, 2 * kp:2 * kp + 2, t * 128:(t + 1) * 128],
                            rhs=xT8[:, 2 * kp:2 * kp + 2, sl],
                            start=(kp == 0), stop=False, perf_mode=DR)
                    for kp in range(NP):
                        nc.tensor.matmul(
                            ps, lhsT=wyr_sb[:, 2 * kp:2 * kp + 2, t * 128:(t + 1) * 128],
                            rhs=yT8[:, 2 * kp:2 * kp + 2, sl],
                            start=False, stop=(kp == NP - 1), perf_mode=DR)
                    rt = p6.tile([128, 512], BF16, tag="rt")
                    nc.scalar.activation(rt, ps, AF.Sigmoid, scale=1.0 / WS)
                    nc.vector.tensor_mul(rx[:, t, sl], rt, xTb[:, t, sl])

        # =========== P7: z/h + gated combine (seq-major, single pass) ========
        with tc.tile_pool(name="p7", bufs=2) as p7:
            for st in range(NT):
                ss = slice(st * 128, (st + 1) * 128)
                xf = p7.tile([128, E], F32, tag="xf")
                nc.sync.dma_start(out=xf, in_=xb_d.ap()[ss, :])
                ot = p7.tile([128, E], F32, tag="ot")
                for c in range(2):
                    sl = slice(c * 512, (c + 1) * 512)
                    zps = psum.tile([128, 512], F32, tag="mm", bufs=3, name="ps_z")
                    for kp in range(NP):
                        nc.tensor.matmul(zps, lhsT=xT8[:, 2 * kp:2 * kp + 2, ss],
                                         rhs=wxzh_sb[:, 2 * kp:2 * kp + 2, sl],
                                         start=(kp == 0), stop=False, perf_mode=DR)
                    for kp in range(NP):
                        nc.tensor.matmul(zps, lhsT=xl8[:, 2 * kp:2 * kp + 2, ss],
                                         rhs=wxzh2_sb[:, 2 * kp:2 * kp + 2, sl],
                                         start=False, stop=False, perf_mode=DR)
                    for kp in range(NP):
                        nc.tensor.matmul(zps, lhsT=xT8[:, 2 * kp:2 * kp + 2, ss],
                                         rhs=wxzr_sb[:, 2 * kp:2 * kp + 2, sl],
                                         start=False, stop=False, perf_mode=DR)
                    for kp in range(NP):
                        nc.tensor.matmul(zps, lhsT=yT8[:, 2 * kp:2 * kp + 2, ss],
                                         rhs=wyz_sb[:, 2 * kp:2 * kp + 2, sl],
                                         start=False, stop=False, perf_mode=DR)
                    nc.tensor.matmul(zps, lhsT=ones1_b, rhs=bxz_row[:, sl],
                                     start=False, stop=True)
                    zt = p7.tile([128, 512], F32, tag="zt")
                    nc.scalar.activation(zt, zps, AF.Sigmoid, scale=1.0 / WS)
                    hps = psum.tile([128, 512], F32, tag="mm", bufs=3, name="ps_h")
                    for kp in range(NP):
                        nc.tensor.matmul(hps, lhsT=rx[:, 2 * kp:2 * kp + 2, ss],
                                         rhs=wxg_sb[:, 2 * kp:2 * kp + 2, sl],
                                         start=(kp == 0), stop=False, perf_mode=DR)
                    for kp in range(NP):
                        nc.tensor.matmul(hps, lhsT=yT8[:, 2 * kp:2 * kp + 2, ss],
                                         rhs=wyg_sb[:, 2 * kp:2 * kp + 2, sl],
                                         start=False, stop=(kp == NP - 1), perf_mode=DR)
                    ht = p7.tile([128, 512], F32, tag="ht")
                    nc.scalar.activation(ht, hps, AF.Tanh, scale=1.0 / WS)
                    dt = p7.tile([128, 512], F32, tag="dt")
                    nc.vector.tensor_sub(dt, ht, xf[:, sl])
                    zd = p7.tile([128, 512], F32, tag="zd")
                    nc.vector.tensor_mul(zd, zt, dt)
                    nc.vector.tensor_add(ot[:, sl], xf[:, sl], zd)
                nc.sync.dma_start(out=out_d.ap()[ss, :], in_=ot)

    nc.compile()
    return nc


# ---------------- host-side packing -----------------------------------------

def _pack_w(w, scale, npdt):
    return np.ascontiguousarray(
        (np.asarray(w, np.float32) * scale).astype(npdt)
        .reshape(NT, 128, E).transpose(1, 0, 2).reshape(128, NT * E))


def _pack_fm(m, npdt):
    # [E, S]-logical feature-major -> [128, NT*S]
    return np.ascontiguousarray(
        m.astype(npdt).reshape(NT, 128, S).transpose(1, 0, 2).reshape(128, NT * S))


def _pack_bias_fm(b, scale=1.0):
    return np.ascontiguousarray(
        (np.asarray(b, np.float32) * scale).reshape(NT, 128).T)


_INV = None


def _inv_pair():
    global _INV
    if _INV is None:
        inv = 1.0 / (10000.0 ** (np.arange(0, E, 2, dtype=np.float32) / np.float32(E)))
        _INV = np.repeat(inv.astype(np.float64), 2)  # pair-expanded [E]
    return _INV


def _tables(idx):
    f = _inv_pair()[:, None] * idx.astype(np.float64)[None, :]  # [E, S]
    return (_pack_fm(np.cos(f).astype(np.float32), NPBF16),
            _pack_fm(np.sin(f).astype(np.float32), NPBF16))


def _pmat():
    pm = np.zeros((128, 128), dtype=NPBF16)
    for i in range(64):
        pm[2 * i + 1, 2 * i] = -1.0
        pm[2 * i, 2 * i + 1] = 1.0
    return pm


def make_in_maps(inputs, share_qk):
    x = np.asarray(inputs["x"], dtype=np.float32)
    qi = np.asarray(inputs["query_index"])
    ki = np.asarray(inputs["key_index"])
    common = {
        "bq": _pack_bias_fm(np.asarray(inputs["bq"])),
        "bk": _pack_bias_fm(np.asarray(inputs["bk"])),
        "bo8": _pack_bias_fm(np.asarray(inputs["bo"]), YS),
        "bv_row": (np.asarray(inputs["bv"], np.float32) * WS).astype(NPBF16).reshape(1, E),
        "bxz_row": (np.asarray(inputs["bxz"], np.float32) * WS).astype(NPBF16).reshape(1, E),
        "ones8": np.full((128, 256), 1.0 / CS, NPFP8),
        "pmat": _pmat(),
    }
    for nm in ("Wq", "Wk", "Wv", "Wo", "Wxr", "Wxg"):
        common[nm] = _pack_w(inputs[nm], WS, NPFP8)
    for nm in ("Wyr", "Wyz", "Wyg"):
        common[nm] = _pack_w(inputs[nm], WS / YS, NPFP8)
    wxz = np.asarray(inputs["Wxz"], np.float32)
    whi8 = (wxz * WS).astype(NPFP8)
    common["Wxzh"] = _pack_w(whi8.astype(np.float32), 1.0, NPFP8)
    common["Wxzh2"] = _pack_w(wxz, 2.0, NPFP8)
    common["WxzR"] = _pack_w(wxz * WS - whi8.astype(np.float32), 1.0, NPFP8)
    in_maps = []
    for b in range(B):
        m = dict(common)
        xb = np.ascontiguousarray(x[b])
        m["xb"] = xb
        xt = xb.T  # [E, S]
        m["xtb"] = _pack_fm(xt, NPBF16)
        xh8 = xt.astype(NPFP8)
        m["xt8"] = _pack_fm(xh8.astype(np.float32), NPFP8)
        m["xl8"] = _pack_fm((xt - xh8.astype(np.float32)) * 16.0, NPFP8)
        m["cosq"], m["sinq"] = _tables(qi[b])
        if not share_qk:
            m["cosk"], m["sink"] = _tables(ki[b])
        in_maps.append(m)
    return in_maps


def kernel(**inputs):
    qi = np.asarray(inputs["query_index"])
    ki = np.asarray(inputs["key_index"])
    share_qk = bool(np.array_equal(qi, ki))

    key = ("k", share_qk)
    if key not in _COMPILED:
        _COMPILED[key] = _build(share_qk)
    nc = _COMPILED[key]

    in_maps = make_in_maps(inputs, share_qk)
    global _dbg_in_maps
    _dbg_in_maps = in_maps
    res = bass_utils.run_bass_kernel_spmd(nc, in_maps, core_ids=list(range(NC)))
    out = np.stack([res.results[b]["out"] for b in range(B)]).astype(np.float32)
    return out
